# revision 9
# baseline (speedup 1.0000x reference)
"""Trainium2 Bass kernel for 2-layer RGCN + query MLP (nn_BeeSender).

Algorithm (per core, dst-sharded):
  out[d] = sum_r (sum_{e->d, rel r} coef_e * x[src_e]) @ W_rel[r]
           + x[d] @ W_root + b
i.e. gather-then-transform: aggregate raw features per (dst, rel) via
one-hot selection matmuls on the tensor engine, then apply per-relation
weights densely.  Edges are routed to the core owning dst (N/8 shard),
sorted by (src-bank, dst-tile, rel), padded to 32-slot quanta with
across-core-equalized capacities so all 8 cores share one SPMD program.
Per-edge rows are fetched with dma_gather (4 SWDGE queues, <=1024
idxs/call, int16 in-bank indices).  h1 is AllGathered between layers;
the final 1024-query MLP is data-parallel (128 queries/core).
"""
import os
import sys

sys.path.insert(0, "/opt/trn_rl_repo")

import numpy as np
import ml_dtypes

from concourse import bass, bacc, mybir
import concourse.tile as tile
from concourse.masks import make_identity
from concourse.bass_utils import run_bass_kernel_spmd

NC = 8
F = 128
R = 4
HID = 256
P = 128

# full-problem config
N_FULL = 100000
NPAD_FULL = 102400
B_FULL = 1024

BANK = 32768
QUANT = 32
CALL_MAX = 1024

DT_TABLE = mybir.dt.bfloat16
NP_TABLE = ml_dtypes.bfloat16
F32 = mybir.dt.float32

_last_exec_ns = None


# ----------------------------------------------------------------- host prep

class Meta:
    pass


def _prep_meta(src, dst, et, npad, n_nodes):
    """Shared (across-core) slot-layout metadata + per-core counts.

    Returns meta with: ncap [NB,T,R] slot capacities, seg_pos [NB,T,R] global
    slot offsets, bank_base/bank_len, S total slots, C chunks, calls list,
    submm[t][r] = list of (callid, ccol, plo, phi, chunk).
    """
    shard = npad // NC
    tiles = shard // P
    nb = (npad + BANK - 1) // BANK

    core = dst // shard
    t_loc = (dst % shard) // P
    b = src // BANK

    # counts per (core, b, t, r)
    lin = ((core * nb + b) * tiles + t_loc) * R + et
    cnt = np.bincount(lin, minlength=NC * nb * tiles * R).reshape(NC, nb, tiles, R)

    # slot capacity per segment = max across cores (no alignment quantum:
    # matmuls always span the full 128-row chunk; non-segment rows are
    # poisoned in the per-piece dst column so they contribute zero)
    ncap = cnt.max(axis=0).astype(np.int64)  # [nb,t,r]

    meta = Meta()
    meta.shard, meta.tiles, meta.nb = shard, tiles, nb
    meta.ncap = ncap
    meta.cnt = cnt

    # bank streams: within bank b, segments ordered (t, r); tail pad to 128
    seg_pos = np.zeros((nb, tiles, R), np.int64)
    bank_base = np.zeros(nb, np.int64)
    bank_len = np.zeros(nb, np.int64)
    pos = 0
    for bb in range(nb):
        bank_base[bb] = pos
        for t in range(tiles):
            for r in range(R):
                seg_pos[bb, t, r] = pos
                pos += int(ncap[bb, t, r])
        pos = (pos + P - 1) // P * P
        bank_len[bb] = pos - bank_base[bb]
    meta.seg_pos, meta.bank_base, meta.bank_len = seg_pos, bank_base, bank_len
    meta.S = int(pos)
    meta.C = meta.S // P

    # calls: windows of <=CALL_MAX slots per bank
    calls = []
    for bb in range(nb):
        s0 = int(bank_base[bb])
        end = s0 + int(bank_len[bb])
        while s0 < end:
            n = min(CALL_MAX, end - s0)
            calls.append(dict(b=bb, slot0=s0, n=n, col0=s0 // 16, chunk0=s0 // P,
                              row0=bb * BANK, rows=min(BANK, npad - bb * BANK)))
            s0 += n
    meta.calls = calls
    # chunk -> call id
    chunk2call = np.zeros(meta.C, np.int64)
    for ci, c in enumerate(calls):
        chunk2call[c["chunk0"]: c["chunk0"] + c["n"] // P] = ci
    meta.chunk2call = chunk2call

    # pieces per (t, r): each piece = (callid, ccol, chunk, piece_col).
    # piece_col indexes the per-piece poisoned dst column array.  Per
    # segment, pieces cover chunks seg_pos//P .. (seg_pos+ncap-1)//P.
    submm = [[[] for _ in range(R)] for _ in range(tiles)]
    piece_col0 = np.zeros((nb, tiles, R), np.int64)  # first piece col per seg
    npieces = 0
    for bb in range(nb):
        for t in range(tiles):
            for r in range(R):
                L = int(ncap[bb, t, r])
                piece_col0[bb, t, r] = npieces
                if L == 0:
                    continue
                pos = int(seg_pos[bb, t, r])
                c0, c1 = pos // P, (pos + L - 1) // P
                for chunk in range(c0, c1 + 1):
                    ci = int(chunk2call[chunk])
                    submm[t][r].append(
                        (ci, chunk - calls[ci]["chunk0"], chunk, npieces))
                    npieces += 1
    meta.submm = submm
    meta.piece_col0 = piece_col0
    meta.npieces = npieces
    return meta


def _prep_core_arrays(meta, src, dst, et, coef, npad, c):
    """Per-core idx/dstloc/coef arrays in the shared slot layout."""
    shard, tiles, nb = meta.shard, meta.tiles, meta.nb
    core = dst // shard
    m = core == c
    s, d, r, cf = src[m], dst[m], et[m], coef[m]
    b = s // BANK
    t_loc = (d % shard) // P

    order = np.lexsort((s, r, t_loc, b))
    s, d, r, cf, b, t_loc = (a[order] for a in (s, d, r, cf, b, t_loc))

    gid = (b * tiles + t_loc) * R + r  # bank-major group id, sorted order
    # rank within group
    n = s.shape[0]
    grp_change = np.concatenate([[True], gid[1:] != gid[:-1]])
    grp_start = np.maximum.accumulate(np.where(grp_change, np.arange(n), 0))
    rank = np.arange(n) - grp_start

    slot = meta.seg_pos.reshape(-1)[gid] + rank

    idx16 = np.zeros((16, meta.S // 16), np.int16)
    idx16[slot % 16, slot // 16] = (s % BANK).astype(np.int16)
    idx_arr = np.tile(idx16, (8, 1))

    # per-piece poisoned dst columns: rows not in the piece keep 999
    # (no iota match -> zero Sel entry); coef is per chunk.
    pcol = (meta.piece_col0.reshape(-1)[gid]
            + slot // P - meta.seg_pos.reshape(-1)[gid] // P)
    dstl = np.full((P, max(meta.npieces, 1)), 999.0, np.float32)
    dstl[slot % P, pcol] = (d % P).astype(np.float32)
    coefa = np.zeros((P, meta.C), np.float32)
    coefa[slot % P, slot // P] = cf
    return idx_arr, dstl, coefa


# ------------------------------------------------------------ program builder

def _build_program(meta, npad, nqueries_per_core):
    shard, tiles = meta.shard, meta.tiles
    dtt = DT_TABLE

    nc = bacc.Bacc("TRN2", target_bir_lowering=False, debug=False,
                   num_swdge_queues=4)

    xpad = nc.declare_dram_parameter("xpad", [npad, F], dtt, isOutput=False)
    xshard = nc.declare_dram_parameter("xshard", [shard, F], dtt, isOutput=False)
    idx_d = nc.declare_dram_parameter("idx", [P, meta.S // 16], mybir.dt.int16, isOutput=False)
    dstl_d = nc.declare_dram_parameter("dstl", [P, max(meta.npieces, 1)], F32, isOutput=False)
    coef_d = nc.declare_dram_parameter("coef", [P, meta.C], F32, isOutput=False)
    iota_d = nc.declare_dram_parameter("iota", [P, P], F32, isOutput=False)
    w1_d = nc.declare_dram_parameter("w1", [F, R * F], dtt, isOutput=False)
    w1r_d = nc.declare_dram_parameter("w1r", [F, F], dtt, isOutput=False)
    b1b_d = nc.declare_dram_parameter("b1b", [P, F], F32, isOutput=False)
    w2_d = nc.declare_dram_parameter("w2", [F, R * F], dtt, isOutput=False)
    w2r_d = nc.declare_dram_parameter("w2r", [F, F], dtt, isOutput=False)
    b2b_d = nc.declare_dram_parameter("b2b", [P, F], F32, isOutput=False)
    wfcn_d = nc.declare_dram_parameter("wfcn", [F, HID], dtt, isOutput=False)
    wfcf_d = nc.declare_dram_parameter("wfcf", [F, HID], dtt, isOutput=False)
    bfc_d = nc.declare_dram_parameter("bfc", [P, HID // P], F32, isOutput=False)
    nest_d = nc.declare_dram_parameter("nest", [nqueries_per_core, 1], mybir.dt.int32, isOutput=False)
    food_d = nc.declare_dram_parameter("food", [nqueries_per_core, 1], mybir.dt.int32, isOutput=False)
    out_d = nc.declare_dram_parameter("out", [HID, nqueries_per_core], F32, isOutput=True)

    h1s = nc.dram_tensor("h1s", [shard, F], dtt)
    h1f = nc.dram_tensor("h1f", [npad, F], dtt)
    h2s = nc.dram_tensor("h2s", [shard, F], dtt)
    h2f = nc.dram_tensor("h2f", [npad, F], dtt)

    with tile.TileContext(nc) as tc:
        with (
            tc.tile_pool(name="const", bufs=1) as cpool,
            tc.tile_pool(name="xg", bufs=16) as xgp,
            tc.tile_pool(name="sel", bufs=12) as selp,
            tc.tile_pool(name="work", bufs=3) as work,
            tc.tile_pool(name="pa", bufs=2, space="PSUM") as psum_a,
            tc.tile_pool(name="pt", bufs=2, space="PSUM") as psum_t,
            tc.tile_pool(name="po", bufs=2, space="PSUM") as psum_o,
        ):
            idx_t = cpool.tile([P, meta.S // 16], mybir.dt.int16)
            dstl_t = cpool.tile([P, max(meta.npieces, 1)], F32)
            coef_t = cpool.tile([P, meta.C], F32)
            iota_t = cpool.tile([P, P], F32)
            w1_t = cpool.tile([F, R * F], dtt)
            w1r_t = cpool.tile([F, F], dtt)
            b1b_t = cpool.tile([P, F], F32)
            w2_t = cpool.tile([F, R * F], dtt)
            w2r_t = cpool.tile([F, F], dtt)
            b2b_t = cpool.tile([P, F], F32)
            wfcn_t = cpool.tile([F, HID], dtt)
            wfcf_t = cpool.tile([F, HID], dtt)
            bfc_t = cpool.tile([P, HID // P], F32)
            nest_t = cpool.tile([nqueries_per_core, 1], mybir.dt.int32)
            food_t = cpool.tile([nqueries_per_core, 1], mybir.dt.int32)
            ident = cpool.tile([P, P], dtt)

            for tt, dd in [(idx_t, idx_d), (dstl_t, dstl_d), (coef_t, coef_d),
                           (iota_t, iota_d), (w1_t, w1_d), (w1r_t, w1r_d),
                           (b1b_t, b1b_d), (w2_t, w2_d), (w2r_t, w2r_d),
                           (b2b_t, b2b_d), (wfcn_t, wfcn_d), (wfcf_t, wfcf_d),
                           (bfc_t, bfc_d), (nest_t, nest_d), (food_t, food_d)]:
                nc.sync.dma_start(out=tt[:], in_=dd[:])
            make_identity(nc, ident[:])

            qctr = [0]

            def layer(table_h, root_h, w_t, wr_t, bb_t, relu, out_h):
                xg_tiles = {}
                for t in range(tiles):
                    at_p = psum_a.tile([P, R * F], F32, space="PSUM", tag="pa")
                    used_r = []
                    for r in range(R):
                        pieces = meta.submm[t][r]
                        if not pieces:
                            continue
                        used_r.append(r)
                        for i, (ci, ccol, chunk, pcol) in enumerate(pieces):
                            if ci not in xg_tiles:
                                call = meta.calls[ci]
                                xt = xgp.tile([P, CALL_MAX // P, F], dtt, tag="xg")
                                nc.gpsimd.dma_gather(
                                    xt[:, : call["n"] // P, :],
                                    table_h[call["row0"]: call["row0"] + call["rows"]],
                                    idx_t[:, call["col0"]: call["col0"] + call["n"] // 16],
                                    call["n"], call["n"], F,
                                    queue_num=qctr[0] % 4,
                                )
                                qctr[0] += 1
                                xg_tiles[ci] = xt
                            st = selp.tile([P, P], dtt, tag="sel")
                            nc.vector.tensor_scalar(
                                out=st[:], in0=iota_t[:],
                                scalar1=dstl_t[:, pcol:pcol + 1],
                                scalar2=coef_t[:, chunk:chunk + 1],
                                op0=mybir.AluOpType.is_equal,
                                op1=mybir.AluOpType.mult,
                            )
                            nc.tensor.matmul(
                                out=at_p[:, r * F:(r + 1) * F],
                                lhsT=xg_tiles[ci][:, ccol, :],
                                rhs=st[:],
                                start=(i == 0), stop=(i == len(pieces) - 1),
                            )
                    at_s = work.tile([P, R * F], dtt, tag="at_s")
                    if used_r:
                        nc.vector.tensor_copy(out=at_s[:], in_=at_p[:])
                    # root rows, transposed via PE
                    xd = work.tile([P, F], dtt, tag="xd")
                    nc.sync.dma_start(out=xd[:], in_=root_h[t * P:(t + 1) * P])
                    xt_p = psum_t.tile([P, F], dtt, space="PSUM", tag="pt")
                    nc.tensor.transpose(out=xt_p[:], in_=xd[:], identity=ident[:])
                    xts = work.tile([P, F], dtt, tag="xts")
                    nc.vector.tensor_copy(out=xts[:], in_=xt_p[:])

                    out_p = psum_o.tile([P, F], F32, space="PSUM", tag="po")
                    mms = [(at_s[:, r * F:(r + 1) * F], w_t[:, r * F:(r + 1) * F])
                           for r in used_r] + [(xts[:], wr_t[:])]
                    for i, (l, rr) in enumerate(mms):
                        nc.tensor.matmul(out=out_p[:], lhsT=l, rhs=rr,
                                         start=(i == 0), stop=(i == len(mms) - 1))
                    ht = work.tile([P, F], dtt, tag="ht")
                    if relu:
                        hsum = work.tile([P, F], F32, tag="hsum")
                        nc.vector.tensor_tensor(out=hsum[:], in0=out_p[:],
                                                in1=bb_t[:], op=mybir.AluOpType.add)
                        nc.vector.tensor_scalar_max(out=ht[:], in0=hsum[:], scalar1=0.0)
                    else:
                        nc.vector.tensor_tensor(out=ht[:], in0=out_p[:],
                                                in1=bb_t[:], op=mybir.AluOpType.add)
                    nc.sync.dma_start(out=out_h[t * P:(t + 1) * P], in_=ht[:])

            layer(xpad, xshard, w1_t, w1r_t, b1b_t, True, h1s)
            nc.gpsimd.collective_compute(
                "AllGather", mybir.AluOpType.bypass,
                replica_groups=[list(range(NC))],
                ins=[h1s[:, :]], outs=[h1f[:, :]],
            )
            layer(h1f, h1s, w2_t, w2r_t, b2b_t, False, h2s)
            nc.gpsimd.collective_compute(
                "AllGather", mybir.AluOpType.bypass,
                replica_groups=[list(range(NC))],
                ins=[h2s[:, :]], outs=[h2f[:, :]],
            )

            # final MLP: gather query rows, transpose, two-chunk matmul
            for name, qt in [("nest", nest_t), ("food", food_t)]:
                g = work.tile([nqueries_per_core, F], dtt, tag=f"q_{name}")
                nc.gpsimd.indirect_dma_start(
                    out=g[:], out_offset=None, in_=h2f[:],
                    in_offset=bass.IndirectOffsetOnAxis(ap=qt[:, :1], axis=0),
                )
                gp = psum_t.tile([P, nqueries_per_core], dtt, space="PSUM", tag="pt")
                nc.tensor.transpose(out=gp[:, :], in_=g[:],
                                    identity=ident[:nqueries_per_core, :nqueries_per_core])
                gs = work.tile([P, nqueries_per_core], dtt, tag=f"qT_{name}")
                nc.vector.tensor_copy(out=gs[:], in_=gp[:])
                if name == "nest":
                    nestT = gs
                else:
                    foodT = gs
            for c in range(HID // P):
                fcp = psum_o.tile([P, nqueries_per_core], F32, space="PSUM", tag="po")
                nc.tensor.matmul(out=fcp[:], lhsT=wfcn_t[:, c * P:(c + 1) * P],
                                 rhs=nestT[:], start=True, stop=False)
                nc.tensor.matmul(out=fcp[:], lhsT=wfcf_t[:, c * P:(c + 1) * P],
                                 rhs=foodT[:], start=False, stop=True)
                fco = work.tile([P, nqueries_per_core], F32, tag="fco")
                nc.vector.tensor_scalar(out=fco[:], in0=fcp[:],
                                        scalar1=bfc_t[:, c:c + 1], scalar2=0.0,
                                        op0=mybir.AluOpType.add,
                                        op1=mybir.AluOpType.max)
                nc.sync.dma_start(out=out_d[c * P:(c + 1) * P, :], in_=fco[:])

    nc.compile()
    return nc


# ------------------------------------------------------------------- kernel()

def _run(x, edge_index, edge_type, nest_idx, food_idx,
         W_rel1, W_root1, b1, W_rel2, W_root2, b2, W_fc, b_fc,
         n_nodes, npad, nb_total):
    global _last_exec_ns
    shard = npad // NC
    src = np.asarray(edge_index[0], np.int64)
    dst = np.asarray(edge_index[1], np.int64)
    et = np.asarray(edge_type, np.int64)
    nest_idx = np.asarray(nest_idx, np.int64)
    food_idx = np.asarray(food_idx, np.int64)

    key = dst * R + et
    deg = np.bincount(key, minlength=n_nodes * R)
    coef = (1.0 / np.maximum(deg, 1.0)[key]).astype(np.float32)

    meta = _prep_meta(src, dst, et, npad, n_nodes)

    x_pad = np.zeros((npad, F), NP_TABLE)
    x_pad[:n_nodes] = np.asarray(x, np.float32).astype(NP_TABLE)
    iota = np.tile(np.arange(P, dtype=np.float32), (P, 1))
    w1 = np.concatenate([np.asarray(W_rel1[r], np.float32) for r in range(R)], axis=1).astype(NP_TABLE)
    w2 = np.concatenate([np.asarray(W_rel2[r], np.float32) for r in range(R)], axis=1).astype(NP_TABLE)
    w1r = np.asarray(W_root1, np.float32).astype(NP_TABLE)
    w2r = np.asarray(W_root2, np.float32).astype(NP_TABLE)
    b1b = np.tile(np.asarray(b1, np.float32), (P, 1))
    b2b = np.tile(np.asarray(b2, np.float32), (P, 1))
    wfc = np.asarray(W_fc, np.float32)
    wfcn = wfc[:F].astype(NP_TABLE)
    wfcf = wfc[F:].astype(NP_TABLE)
    bfc = np.asarray(b_fc, np.float32).reshape(HID // P, P).T.copy()

    nq = nb_total // NC
    in_maps = []
    for c in range(NC):
        idx_arr, dstl, coefa = _prep_core_arrays(meta, src, dst, et, coef, npad, c)
        in_maps.append(dict(
            xpad=x_pad,
            xshard=np.ascontiguousarray(x_pad[c * shard:(c + 1) * shard]),
            idx=idx_arr, dstl=dstl, coef=coefa, iota=iota,
            w1=w1, w1r=w1r, b1b=b1b, w2=w2, w2r=w2r, b2b=b2b,
            wfcn=wfcn, wfcf=wfcf, bfc=bfc,
            nest=nest_idx[c * nq:(c + 1) * nq].astype(np.int32)[:, None],
            food=food_idx[c * nq:(c + 1) * nq].astype(np.int32)[:, None],
        ))

    nc = _build_program(meta, npad, nq)

    trace = bool(os.environ.get("KERNEL_PROFILE"))
    res = run_bass_kernel_spmd(nc, in_maps, list(range(NC)), trace=trace)
    if trace:
        _last_exec_ns = res.exec_time_ns

    out = np.empty((nb_total, HID), np.float32)
    for c in range(NC):
        out[c * nq:(c + 1) * nq] = res.results[c]["out"].T
    return out


def kernel(x, edge_index, edge_type, nest_idx, food_idx,
           W_rel1, W_root1, b1, W_rel2, W_root2, b2, W_fc, b_fc):
    return _run(x, edge_index, edge_type, nest_idx, food_idx,
                W_rel1, W_root1, b1, W_rel2, W_root2, b2, W_fc, b_fc,
                n_nodes=N_FULL, npad=NPAD_FULL, nb_total=B_FULL)


# ------------------------------------------------------------------- selftest

def _np_reference(x, src, dst, et, nest, food, W_rel1, W_root1, b1,
                  W_rel2, W_root2, b2, W_fc, b_fc, n):
    def conv(h, W_rel, W_root, b):
        hr = np.einsum("nf,rfo->nro", h, W_rel)
        msgs = hr[src, et]
        key = dst * R + et
        deg = np.bincount(key, minlength=n * R).astype(np.float32)
        norm = 1.0 / np.maximum(deg, 1.0)
        out = np.zeros((n, W_rel.shape[2]), np.float32)
        np.add.at(out, dst, msgs * norm[key][:, None])
        return out + h @ W_root + b

    h = np.maximum(conv(x, W_rel1, W_root1, b1), 0.0)
    h = conv(h, W_rel2, W_root2, b2)
    comb = np.concatenate([h[nest], h[food]], axis=1)
    return np.maximum(comb @ W_fc + b_fc, 0.0)


def _selftest():
    global BANK
    BANK = 1024  # exercise multi-bank path at small scale
    n, npad, nb = 4000, 4096, 256
    e = 32768
    rng = np.random.default_rng(0)
    x = rng.standard_normal((n, F)).astype(np.float32)
    ei = rng.integers(0, n, (2, e)).astype(np.int64)
    et = rng.integers(0, R, e).astype(np.int64)
    nest = rng.integers(0, n, nb).astype(np.int64)
    food = rng.integers(0, n, nb).astype(np.int64)
    s1, s2, sf = 1 / np.sqrt(F), 1 / np.sqrt(F), 1 / np.sqrt(2 * F)
    W_rel1 = (rng.standard_normal((R, F, F)) * s1).astype(np.float32)
    W_root1 = (rng.standard_normal((F, F)) * s1).astype(np.float32)
    b1 = rng.standard_normal(F).astype(np.float32) * 0.1
    W_rel2 = (rng.standard_normal((R, F, F)) * s2).astype(np.float32)
    W_root2 = (rng.standard_normal((F, F)) * s2).astype(np.float32)
    b2 = rng.standard_normal(F).astype(np.float32) * 0.1
    W_fc = (rng.standard_normal((2 * F, HID)) * sf).astype(np.float32)
    b_fc = rng.standard_normal(HID).astype(np.float32) * 0.1

    exp = _np_reference(x, ei[0], ei[1], et, nest, food, W_rel1, W_root1, b1,
                        W_rel2, W_root2, b2, W_fc, b_fc, n)
    act = _run(x, ei, et, nest, food, W_rel1, W_root1, b1,
               W_rel2, W_root2, b2, W_fc, b_fc,
               n_nodes=n, npad=npad, nb_total=nb)
    denom = np.abs(exp).max()
    err = np.abs(act - exp).max() / denom
    print(f"selftest: rel_err={err:.2e} (absmax denom {denom:.3f})")
    assert err < 2e-2, "selftest FAILED"
    print("SELFTEST PASSED")


if __name__ == "__main__":
    _selftest()


# revision 10
# speedup vs baseline: 1.3803x; 1.3803x over previous
"""Trainium2 Bass kernel for 2-layer RGCN + query MLP (nn_BeeSender).

Algorithm (per core, dst-sharded):
  out[d] = sum_r (sum_{e->d, rel r} coef_e * x[src_e]) @ W_rel[r]
           + x[d] @ W_root + b
i.e. gather-then-transform: aggregate raw features per (dst, rel) via
one-hot selection matmuls on the tensor engine, then apply per-relation
weights densely.  Edges are routed to the core owning dst (N/8 shard),
sorted by (src-bank, dst-tile, rel), padded to 32-slot quanta with
across-core-equalized capacities so all 8 cores share one SPMD program.
Per-edge rows are fetched with dma_gather (4 SWDGE queues, <=1024
idxs/call, int16 in-bank indices).  h1 is AllGathered between layers;
the final 1024-query MLP is data-parallel (128 queries/core).
"""
import os
import sys

sys.path.insert(0, "/opt/trn_rl_repo")

import numpy as np
import ml_dtypes

from concourse import bass, bacc, mybir
import concourse.tile as tile
from concourse.masks import make_identity
from concourse.bass_utils import run_bass_kernel_spmd

NC = 8
F = 128
R = 4
HID = 256
P = 128

# full-problem config
N_FULL = 100000
NPAD_FULL = 102400
B_FULL = 1024

BANK = 32768
QUANT = 32
CALL_MAX = 1024

DT_TABLE = mybir.dt.bfloat16
NP_TABLE = ml_dtypes.bfloat16
F32 = mybir.dt.float32

_last_exec_ns = None


# ----------------------------------------------------------------- host prep

class Meta:
    pass


def _prep_meta(src, dst, et, npad, n_nodes):
    """Shared (across-core) slot-layout metadata + per-core counts.

    Returns meta with: ncap [NB,T,R] slot capacities, seg_pos [NB,T,R] global
    slot offsets, bank_base/bank_len, S total slots, C chunks, calls list,
    submm[t][r] = list of (callid, ccol, plo, phi, chunk).
    """
    shard = npad // NC
    tiles = shard // P
    nb = (npad + BANK - 1) // BANK

    core = dst // shard
    t_loc = (dst % shard) // P
    b = src // BANK

    # counts per (core, b, t, r)
    lin = ((core * nb + b) * tiles + t_loc) * R + et
    cnt = np.bincount(lin, minlength=NC * nb * tiles * R).reshape(NC, nb, tiles, R)

    # slot capacity per segment = max across cores (no alignment quantum:
    # matmuls always span the full 128-row chunk; non-segment rows are
    # poisoned in the per-piece dst column so they contribute zero)
    ncap = cnt.max(axis=0).astype(np.int64)  # [nb,t,r]

    meta = Meta()
    meta.shard, meta.tiles, meta.nb = shard, tiles, nb
    meta.ncap = ncap
    meta.cnt = cnt

    # bank streams: within bank b, segments ordered (t, r); tail pad to 128
    seg_pos = np.zeros((nb, tiles, R), np.int64)
    bank_base = np.zeros(nb, np.int64)
    bank_len = np.zeros(nb, np.int64)
    pos = 0
    for bb in range(nb):
        bank_base[bb] = pos
        for t in range(tiles):
            for r in range(R):
                seg_pos[bb, t, r] = pos
                pos += int(ncap[bb, t, r])
        pos = (pos + P - 1) // P * P
        bank_len[bb] = pos - bank_base[bb]
    meta.seg_pos, meta.bank_base, meta.bank_len = seg_pos, bank_base, bank_len
    meta.S = int(pos)
    meta.C = meta.S // P

    # calls: windows of <=CALL_MAX slots per bank
    calls = []
    for bb in range(nb):
        s0 = int(bank_base[bb])
        end = s0 + int(bank_len[bb])
        while s0 < end:
            n = min(CALL_MAX, end - s0)
            calls.append(dict(b=bb, slot0=s0, n=n, col0=s0 // 16, chunk0=s0 // P,
                              row0=bb * BANK, rows=min(BANK, npad - bb * BANK)))
            s0 += n
    meta.calls = calls
    # chunk -> call id
    chunk2call = np.zeros(meta.C, np.int64)
    for ci, c in enumerate(calls):
        chunk2call[c["chunk0"]: c["chunk0"] + c["n"] // P] = ci
    meta.chunk2call = chunk2call

    # pieces per (t, r): each piece = (callid, ccol, chunk, piece_col).
    # piece_col indexes the per-piece poisoned dst column array.  Per
    # segment, pieces cover chunks seg_pos//P .. (seg_pos+ncap-1)//P.
    submm = [[[] for _ in range(R)] for _ in range(tiles)]
    piece_col0 = np.zeros((nb, tiles, R), np.int64)  # first piece col per seg
    tile_pcol0 = np.zeros(tiles + 1, np.int64)
    npieces = 0
    for t in range(tiles):
        tile_pcol0[t] = npieces
        for r in range(R):
            for bb in range(nb):
                L = int(ncap[bb, t, r])
                piece_col0[bb, t, r] = npieces
                if L == 0:
                    continue
                pos = int(seg_pos[bb, t, r])
                c0, c1 = pos // P, (pos + L - 1) // P
                for chunk in range(c0, c1 + 1):
                    ci = int(chunk2call[chunk])
                    submm[t][r].append(
                        (ci, chunk - calls[ci]["chunk0"], chunk, npieces))
                    npieces += 1
    tile_pcol0[tiles] = npieces
    meta.submm = submm
    meta.piece_col0 = piece_col0
    meta.tile_pcol0 = tile_pcol0
    meta.npieces = npieces
    return meta


def _prep_core_arrays(meta, src, dst, et, coef, npad, c):
    """Per-core idx/dstloc/coef arrays in the shared slot layout."""
    shard, tiles, nb = meta.shard, meta.tiles, meta.nb
    core = dst // shard
    m = core == c
    s, d, r, cf = src[m], dst[m], et[m], coef[m]
    b = s // BANK
    t_loc = (d % shard) // P

    order = np.lexsort((s, r, t_loc, b))
    s, d, r, cf, b, t_loc = (a[order] for a in (s, d, r, cf, b, t_loc))

    gid = (b * tiles + t_loc) * R + r  # bank-major group id, sorted order
    # rank within group
    n = s.shape[0]
    grp_change = np.concatenate([[True], gid[1:] != gid[:-1]])
    grp_start = np.maximum.accumulate(np.where(grp_change, np.arange(n), 0))
    rank = np.arange(n) - grp_start

    slot = meta.seg_pos.reshape(-1)[gid] + rank

    idx16 = np.zeros((16, meta.S // 16), np.int16)
    idx16[slot % 16, slot // 16] = (s % BANK).astype(np.int16)
    idx_arr = np.tile(idx16, (8, 1))

    # per-piece poisoned dst columns: rows not in the piece keep 999
    # (no iota match -> zero Sel entry); coef is per chunk.
    pc0 = meta.piece_col0.transpose(1, 2, 0).reshape(-1)  # (t,r,b) order
    gid_trb = (t_loc * R + r) * nb + b
    pcol = (pc0[gid_trb]
            + slot // P - meta.seg_pos.reshape(-1)[gid] // P)
    dstl = np.full((P, max(meta.npieces, 1), 1), 999.0, ml_dtypes.bfloat16)
    dstl[slot % P, pcol, 0] = (d % P).astype(ml_dtypes.bfloat16)
    coefa = np.zeros((P, meta.C, 1), ml_dtypes.bfloat16)
    coefa[slot % P, slot // P, 0] = cf.astype(ml_dtypes.bfloat16)
    return idx_arr, dstl, coefa


# ------------------------------------------------------------ program builder

def _build_program(meta, npad, nqueries_per_core):
    shard, tiles = meta.shard, meta.tiles
    dtt = DT_TABLE

    nc = bacc.Bacc("TRN2", target_bir_lowering=False, debug=False,
                   num_swdge_queues=4)

    xpad = nc.declare_dram_parameter("xpad", [npad, F], dtt, isOutput=False)
    xshard = nc.declare_dram_parameter("xshard", [shard, F], dtt, isOutput=False)
    idx_d = nc.declare_dram_parameter("idx", [P, meta.S // 16], mybir.dt.int16, isOutput=False)
    dstl_d = nc.declare_dram_parameter("dstl", [P, max(meta.npieces, 1), 1], dtt, isOutput=False)
    coef_d = nc.declare_dram_parameter("coef", [P, meta.C, 1], dtt, isOutput=False)
    iota_d = nc.declare_dram_parameter("iota", [P, 1, P], dtt, isOutput=False)
    w1_d = nc.declare_dram_parameter("w1", [F, R * F], dtt, isOutput=False)
    w1r_d = nc.declare_dram_parameter("w1r", [F, F], dtt, isOutput=False)
    b1b_d = nc.declare_dram_parameter("b1b", [P, F], F32, isOutput=False)
    w2_d = nc.declare_dram_parameter("w2", [F, R * F], dtt, isOutput=False)
    w2r_d = nc.declare_dram_parameter("w2r", [F, F], dtt, isOutput=False)
    b2b_d = nc.declare_dram_parameter("b2b", [P, F], F32, isOutput=False)
    wfcn_d = nc.declare_dram_parameter("wfcn", [F, HID], dtt, isOutput=False)
    wfcf_d = nc.declare_dram_parameter("wfcf", [F, HID], dtt, isOutput=False)
    bfc_d = nc.declare_dram_parameter("bfc", [P, HID // P], F32, isOutput=False)
    nest_d = nc.declare_dram_parameter("nest", [nqueries_per_core, 1], mybir.dt.int32, isOutput=False)
    food_d = nc.declare_dram_parameter("food", [nqueries_per_core, 1], mybir.dt.int32, isOutput=False)
    out_d = nc.declare_dram_parameter("out", [HID, nqueries_per_core], F32, isOutput=True)

    h1s = nc.dram_tensor("h1s", [shard, F], dtt)
    h1f = nc.dram_tensor("h1f", [npad, F], dtt, addr_space="Shared")
    h2s = nc.dram_tensor("h2s", [shard, F], dtt)
    h2f = nc.dram_tensor("h2f", [npad, F], dtt, addr_space="Shared")

    with tile.TileContext(nc) as tc:
        with (
            tc.tile_pool(name="const", bufs=1) as cpool,
            tc.tile_pool(name="xg", bufs=16) as xgp,
            tc.tile_pool(name="sel", bufs=12) as selp,
            tc.tile_pool(name="work", bufs=3) as work,
            tc.tile_pool(name="pa", bufs=2, space="PSUM") as psum_a,
            tc.tile_pool(name="pt", bufs=2, space="PSUM") as psum_t,
            tc.tile_pool(name="po", bufs=2, space="PSUM") as psum_o,
        ):
            idx_t = cpool.tile([P, meta.S // 16], mybir.dt.int16)
            dstl_t = cpool.tile([P, max(meta.npieces, 1), 1], dtt)
            coef_t = cpool.tile([P, meta.C, 1], dtt)
            iota_t = cpool.tile([P, 1, P], dtt)
            w1_t = cpool.tile([F, R * F], dtt)
            w1r_t = cpool.tile([F, F], dtt)
            b1b_t = cpool.tile([P, F], F32)
            w2_t = cpool.tile([F, R * F], dtt)
            w2r_t = cpool.tile([F, F], dtt)
            b2b_t = cpool.tile([P, F], F32)
            wfcn_t = cpool.tile([F, HID], dtt)
            wfcf_t = cpool.tile([F, HID], dtt)
            bfc_t = cpool.tile([P, HID // P], F32)
            nest_t = cpool.tile([nqueries_per_core, 1], mybir.dt.int32)
            food_t = cpool.tile([nqueries_per_core, 1], mybir.dt.int32)
            ident = cpool.tile([P, P], dtt)

            for tt, dd in [(idx_t, idx_d), (dstl_t, dstl_d), (coef_t, coef_d),
                           (iota_t, iota_d), (w1_t, w1_d), (w1r_t, w1r_d),
                           (b1b_t, b1b_d), (w2_t, w2_d), (w2r_t, w2r_d),
                           (b2b_t, b2b_d), (wfcn_t, wfcn_d), (wfcf_t, wfcf_d),
                           (bfc_t, bfc_d), (nest_t, nest_d), (food_t, food_d)]:
                nc.sync.dma_start(out=tt[:], in_=dd[:])
            make_identity(nc, ident[:])

            qctr = [0]

            def layer(table_h, root_h, w_t, wr_t, bb_t, relu, out_h):
                xg_tiles = {}

                def emit_call(ci):
                    call = meta.calls[ci]
                    ncol = call["n"] // P
                    xt = xgp.tile([P, CALL_MAX // P, F], dtt, tag="xg")
                    nc.gpsimd.dma_gather(
                        xt[:, :ncol, :],
                        table_h[call["row0"]: call["row0"] + call["rows"]],
                        idx_t[:, call["col0"]: call["col0"] + call["n"] // 16],
                        call["n"], call["n"], F,
                        queue_num=qctr[0] % 4,
                    )
                    qctr[0] += 1
                    # fold mean-normalization coefs into the gathered rows
                    c0 = call["chunk0"]
                    nc.vector.tensor_tensor(
                        out=xt[:, :ncol, :], in0=xt[:, :ncol, :],
                        in1=coef_t[:, c0:c0 + ncol, :].to_broadcast([P, ncol, F]),
                        op=mybir.AluOpType.mult,
                    )
                    xg_tiles[ci] = xt

                for t in range(tiles):
                    # one batched Sel build for all of this tile's pieces
                    k0, k1 = int(meta.tile_pcol0[t]), int(meta.tile_pcol0[t + 1])
                    kt = k1 - k0
                    if kt > 0:
                        selb = selp.tile([P, max(kt, 1), P], dtt, tag="sel")
                        nc.vector.tensor_tensor(
                            out=selb[:, :kt, :],
                            in0=dstl_t[:, k0:k1, :].to_broadcast([P, kt, P]),
                            in1=iota_t[:].to_broadcast([P, kt, P]),
                            op=mybir.AluOpType.is_equal,
                        )
                    at_p = psum_a.tile([P, R * F], F32, space="PSUM", tag="pa")
                    used_r = []
                    for r in range(R):
                        pieces = meta.submm[t][r]
                        if not pieces:
                            continue
                        used_r.append(r)
                        for i, (ci, ccol, chunk, pcol) in enumerate(pieces):
                            if ci not in xg_tiles:
                                emit_call(ci)
                            nc.tensor.matmul(
                                out=at_p[:, r * F:(r + 1) * F],
                                lhsT=xg_tiles[ci][:, ccol, :],
                                rhs=selb[:, pcol - k0, :],
                                start=(i == 0), stop=(i == len(pieces) - 1),
                            )
                    at_s = work.tile([P, R * F], dtt, tag="at_s")
                    if used_r:
                        nc.vector.tensor_copy(out=at_s[:], in_=at_p[:])
                    # root rows, transposed via PE
                    xd = work.tile([P, F], dtt, tag="xd")
                    nc.sync.dma_start(out=xd[:], in_=root_h[t * P:(t + 1) * P])
                    xt_p = psum_t.tile([P, F], dtt, space="PSUM", tag="pt")
                    nc.tensor.transpose(out=xt_p[:], in_=xd[:], identity=ident[:])
                    xts = work.tile([P, F], dtt, tag="xts")
                    nc.vector.tensor_copy(out=xts[:], in_=xt_p[:])

                    out_p = psum_o.tile([P, F], F32, space="PSUM", tag="po")
                    mms = [(at_s[:, r * F:(r + 1) * F], w_t[:, r * F:(r + 1) * F])
                           for r in used_r] + [(xts[:], wr_t[:])]
                    for i, (l, rr) in enumerate(mms):
                        nc.tensor.matmul(out=out_p[:], lhsT=l, rhs=rr,
                                         start=(i == 0), stop=(i == len(mms) - 1))
                    ht = work.tile([P, F], dtt, tag="ht")
                    if relu:
                        hsum = work.tile([P, F], F32, tag="hsum")
                        nc.vector.tensor_tensor(out=hsum[:], in0=out_p[:],
                                                in1=bb_t[:], op=mybir.AluOpType.add)
                        nc.vector.tensor_scalar_max(out=ht[:], in0=hsum[:], scalar1=0.0)
                    else:
                        nc.vector.tensor_tensor(out=ht[:], in0=out_p[:],
                                                in1=bb_t[:], op=mybir.AluOpType.add)
                    nc.sync.dma_start(out=out_h[t * P:(t + 1) * P], in_=ht[:])

            layer(xpad, xshard, w1_t, w1r_t, b1b_t, True, h1s)
            nc.gpsimd.collective_compute(
                "AllGather", mybir.AluOpType.bypass,
                replica_groups=[list(range(NC))],
                ins=[h1s[:, :]], outs=[h1f[:, :]],
            )
            layer(h1f, h1s, w2_t, w2r_t, b2b_t, False, h2s)
            nc.gpsimd.collective_compute(
                "AllGather", mybir.AluOpType.bypass,
                replica_groups=[list(range(NC))],
                ins=[h2s[:, :]], outs=[h2f[:, :]],
            )

            # final MLP: gather query rows, transpose, two-chunk matmul
            for name, qt in [("nest", nest_t), ("food", food_t)]:
                g = work.tile([nqueries_per_core, F], dtt, tag=f"q_{name}")
                nc.gpsimd.indirect_dma_start(
                    out=g[:], out_offset=None, in_=h2f[:],
                    in_offset=bass.IndirectOffsetOnAxis(ap=qt[:, :1], axis=0),
                )
                gp = psum_t.tile([P, nqueries_per_core], dtt, space="PSUM", tag="pt")
                nc.tensor.transpose(out=gp[:, :], in_=g[:],
                                    identity=ident[:nqueries_per_core, :nqueries_per_core])
                gs = work.tile([P, nqueries_per_core], dtt, tag=f"qT_{name}")
                nc.vector.tensor_copy(out=gs[:], in_=gp[:])
                if name == "nest":
                    nestT = gs
                else:
                    foodT = gs
            for c in range(HID // P):
                fcp = psum_o.tile([P, nqueries_per_core], F32, space="PSUM", tag="po")
                nc.tensor.matmul(out=fcp[:], lhsT=wfcn_t[:, c * P:(c + 1) * P],
                                 rhs=nestT[:], start=True, stop=False)
                nc.tensor.matmul(out=fcp[:], lhsT=wfcf_t[:, c * P:(c + 1) * P],
                                 rhs=foodT[:], start=False, stop=True)
                fco = work.tile([P, nqueries_per_core], F32, tag="fco")
                nc.vector.tensor_scalar(out=fco[:], in0=fcp[:],
                                        scalar1=bfc_t[:, c:c + 1], scalar2=0.0,
                                        op0=mybir.AluOpType.add,
                                        op1=mybir.AluOpType.max)
                nc.sync.dma_start(out=out_d[c * P:(c + 1) * P, :], in_=fco[:])

    nc.compile()
    return nc


# ------------------------------------------------------------------- kernel()

def _run(x, edge_index, edge_type, nest_idx, food_idx,
         W_rel1, W_root1, b1, W_rel2, W_root2, b2, W_fc, b_fc,
         n_nodes, npad, nb_total):
    global _last_exec_ns
    shard = npad // NC
    src = np.asarray(edge_index[0], np.int64)
    dst = np.asarray(edge_index[1], np.int64)
    et = np.asarray(edge_type, np.int64)
    nest_idx = np.asarray(nest_idx, np.int64)
    food_idx = np.asarray(food_idx, np.int64)

    key = dst * R + et
    deg = np.bincount(key, minlength=n_nodes * R)
    coef = (1.0 / np.maximum(deg, 1.0)[key]).astype(np.float32)

    meta = _prep_meta(src, dst, et, npad, n_nodes)

    x_pad = np.zeros((npad, F), NP_TABLE)
    x_pad[:n_nodes] = np.asarray(x, np.float32).astype(NP_TABLE)
    iota = np.tile(np.arange(P, dtype=np.float32), (P, 1)).astype(
        ml_dtypes.bfloat16).reshape(P, 1, P)
    w1 = np.concatenate([np.asarray(W_rel1[r], np.float32) for r in range(R)], axis=1).astype(NP_TABLE)
    w2 = np.concatenate([np.asarray(W_rel2[r], np.float32) for r in range(R)], axis=1).astype(NP_TABLE)
    w1r = np.asarray(W_root1, np.float32).astype(NP_TABLE)
    w2r = np.asarray(W_root2, np.float32).astype(NP_TABLE)
    b1b = np.tile(np.asarray(b1, np.float32), (P, 1))
    b2b = np.tile(np.asarray(b2, np.float32), (P, 1))
    wfc = np.asarray(W_fc, np.float32)
    wfcn = wfc[:F].astype(NP_TABLE)
    wfcf = wfc[F:].astype(NP_TABLE)
    bfc = np.asarray(b_fc, np.float32).reshape(HID // P, P).T.copy()

    nq = nb_total // NC
    in_maps = []
    for c in range(NC):
        idx_arr, dstl, coefa = _prep_core_arrays(meta, src, dst, et, coef, npad, c)
        in_maps.append(dict(
            xpad=x_pad,
            xshard=np.ascontiguousarray(x_pad[c * shard:(c + 1) * shard]),
            idx=idx_arr, dstl=dstl, coef=coefa, iota=iota,
            w1=w1, w1r=w1r, b1b=b1b, w2=w2, w2r=w2r, b2b=b2b,
            wfcn=wfcn, wfcf=wfcf, bfc=bfc,
            nest=nest_idx[c * nq:(c + 1) * nq].astype(np.int32)[:, None],
            food=food_idx[c * nq:(c + 1) * nq].astype(np.int32)[:, None],
        ))

    nc = _build_program(meta, npad, nq)

    trace = bool(os.environ.get("KERNEL_PROFILE"))
    res = run_bass_kernel_spmd(nc, in_maps, list(range(NC)), trace=trace)
    if trace:
        _last_exec_ns = res.exec_time_ns

    out = np.empty((nb_total, HID), np.float32)
    for c in range(NC):
        out[c * nq:(c + 1) * nq] = res.results[c]["out"].T
    return out


def kernel(x, edge_index, edge_type, nest_idx, food_idx,
           W_rel1, W_root1, b1, W_rel2, W_root2, b2, W_fc, b_fc):
    return _run(x, edge_index, edge_type, nest_idx, food_idx,
                W_rel1, W_root1, b1, W_rel2, W_root2, b2, W_fc, b_fc,
                n_nodes=N_FULL, npad=NPAD_FULL, nb_total=B_FULL)


# ------------------------------------------------------------------- selftest

def _np_reference(x, src, dst, et, nest, food, W_rel1, W_root1, b1,
                  W_rel2, W_root2, b2, W_fc, b_fc, n):
    def conv(h, W_rel, W_root, b):
        hr = np.einsum("nf,rfo->nro", h, W_rel)
        msgs = hr[src, et]
        key = dst * R + et
        deg = np.bincount(key, minlength=n * R).astype(np.float32)
        norm = 1.0 / np.maximum(deg, 1.0)
        out = np.zeros((n, W_rel.shape[2]), np.float32)
        np.add.at(out, dst, msgs * norm[key][:, None])
        return out + h @ W_root + b

    h = np.maximum(conv(x, W_rel1, W_root1, b1), 0.0)
    h = conv(h, W_rel2, W_root2, b2)
    comb = np.concatenate([h[nest], h[food]], axis=1)
    return np.maximum(comb @ W_fc + b_fc, 0.0)


def _selftest():
    global BANK
    BANK = 1024  # exercise multi-bank path at small scale
    n, npad, nb = 4000, 4096, 256
    e = 32768
    rng = np.random.default_rng(0)
    x = rng.standard_normal((n, F)).astype(np.float32)
    ei = rng.integers(0, n, (2, e)).astype(np.int64)
    et = rng.integers(0, R, e).astype(np.int64)
    nest = rng.integers(0, n, nb).astype(np.int64)
    food = rng.integers(0, n, nb).astype(np.int64)
    s1, s2, sf = 1 / np.sqrt(F), 1 / np.sqrt(F), 1 / np.sqrt(2 * F)
    W_rel1 = (rng.standard_normal((R, F, F)) * s1).astype(np.float32)
    W_root1 = (rng.standard_normal((F, F)) * s1).astype(np.float32)
    b1 = rng.standard_normal(F).astype(np.float32) * 0.1
    W_rel2 = (rng.standard_normal((R, F, F)) * s2).astype(np.float32)
    W_root2 = (rng.standard_normal((F, F)) * s2).astype(np.float32)
    b2 = rng.standard_normal(F).astype(np.float32) * 0.1
    W_fc = (rng.standard_normal((2 * F, HID)) * sf).astype(np.float32)
    b_fc = rng.standard_normal(HID).astype(np.float32) * 0.1

    exp = _np_reference(x, ei[0], ei[1], et, nest, food, W_rel1, W_root1, b1,
                        W_rel2, W_root2, b2, W_fc, b_fc, n)
    act = _run(x, ei, et, nest, food, W_rel1, W_root1, b1,
               W_rel2, W_root2, b2, W_fc, b_fc,
               n_nodes=n, npad=npad, nb_total=nb)
    denom = np.abs(exp).max()
    err = np.abs(act - exp).max() / denom
    print(f"selftest: rel_err={err:.2e} (absmax denom {denom:.3f})")
    assert err < 2e-2, "selftest FAILED"
    print("SELFTEST PASSED")


if __name__ == "__main__":
    _selftest()


# revision 16
# speedup vs baseline: 1.3876x; 1.0052x over previous
"""Trainium2 Bass kernel for 2-layer RGCN + query MLP (nn_BeeSender).

Algorithm (per core, dst-sharded):
  out[d] = sum_r (sum_{e->d, rel r} coef_e * x[src_e]) @ W_rel[r]
           + x[d] @ W_root + b
i.e. gather-then-transform: aggregate raw features per (dst, rel) via
one-hot selection matmuls on the tensor engine, then apply per-relation
weights densely.  Edges are routed to the core owning dst (N/8 shard),
sorted by (src-bank, dst-tile, rel), padded to 32-slot quanta with
across-core-equalized capacities so all 8 cores share one SPMD program.
Per-edge rows are fetched with dma_gather (4 SWDGE queues, <=1024
idxs/call, int16 in-bank indices).  h1 is AllGathered between layers;
the final 1024-query MLP is data-parallel (128 queries/core).
"""
import os
import sys

sys.path.insert(0, "/opt/trn_rl_repo")

import numpy as np
import ml_dtypes

from concourse import bass, bacc, mybir
import concourse.tile as tile
from concourse.masks import make_identity
from concourse.bass_utils import run_bass_kernel_spmd

NC = 8
F = 128
R = 4
HID = 256
P = 128

# full-problem config
N_FULL = 100000
NPAD_FULL = 102400
B_FULL = 1024

BANK = 32768
QUANT = 32
CALL_MAX = 1024


def AGCH_FOR(tiles):
    return 4 if tiles % 4 == 0 else 1

DT_TABLE = mybir.dt.bfloat16
NP_TABLE = ml_dtypes.bfloat16
F32 = mybir.dt.float32

_last_exec_ns = None


# ----------------------------------------------------------------- host prep

class Meta:
    pass


def _prep_meta(src, dst, et, npad, n_nodes):
    """Shared (across-core) slot-layout metadata + per-core counts.

    Returns meta with: ncap [NB,T,R] slot capacities, seg_pos [NB,T,R] global
    slot offsets, bank_base/bank_len, S total slots, C chunks, calls list,
    submm[t][r] = list of (callid, ccol, plo, phi, chunk).
    """
    shard = npad // NC
    tiles = shard // P
    nb = (npad + BANK - 1) // BANK

    core = dst // shard
    t_loc = (dst % shard) // P
    b = src // BANK

    # counts per (core, b, t, r)
    lin = ((core * nb + b) * tiles + t_loc) * R + et
    cnt = np.bincount(lin, minlength=NC * nb * tiles * R).reshape(NC, nb, tiles, R)

    # slot capacity per segment = max across cores (no alignment quantum:
    # matmuls always span the full 128-row chunk; non-segment rows are
    # poisoned in the per-piece dst column so they contribute zero)
    ncap = cnt.max(axis=0).astype(np.int64)  # [nb,t,r]

    meta = Meta()
    meta.shard, meta.tiles, meta.nb = shard, tiles, nb
    meta.ncap = ncap
    meta.cnt = cnt

    # bank streams: within bank b, segments ordered (t, r); tail pad to 128
    seg_pos = np.zeros((nb, tiles, R), np.int64)
    bank_base = np.zeros(nb, np.int64)
    bank_len = np.zeros(nb, np.int64)
    pos = 0
    for bb in range(nb):
        bank_base[bb] = pos
        for t in range(tiles):
            for r in range(R):
                seg_pos[bb, t, r] = pos
                pos += int(ncap[bb, t, r])
        pos = (pos + P - 1) // P * P
        bank_len[bb] = pos - bank_base[bb]
    meta.seg_pos, meta.bank_base, meta.bank_len = seg_pos, bank_base, bank_len
    meta.S = int(pos)
    meta.C = meta.S // P

    # calls: windows of <=CALL_MAX slots per bank
    calls = []
    for bb in range(nb):
        s0 = int(bank_base[bb])
        end = s0 + int(bank_len[bb])
        while s0 < end:
            n = min(CALL_MAX, end - s0)
            calls.append(dict(b=bb, slot0=s0, n=n, col0=s0 // 16, chunk0=s0 // P,
                              row0=bb * BANK, rows=min(BANK, npad - bb * BANK)))
            s0 += n
    meta.calls = calls
    # chunk -> call id
    chunk2call = np.zeros(meta.C, np.int64)
    for ci, c in enumerate(calls):
        chunk2call[c["chunk0"]: c["chunk0"] + c["n"] // P] = ci
    meta.chunk2call = chunk2call

    # pieces per (t, r): each piece = (callid, ccol, chunk, piece_col).
    # piece_col indexes the per-piece poisoned dst column array.  Per
    # segment, pieces cover chunks seg_pos//P .. (seg_pos+ncap-1)//P.
    submm = [[[] for _ in range(R)] for _ in range(tiles)]
    piece_col0 = np.zeros((nb, tiles, R), np.int64)  # first piece col per seg
    tile_pcol0 = np.zeros(tiles + 1, np.int64)
    npieces = 0
    for t in range(tiles):
        tile_pcol0[t] = npieces
        for r in range(R):
            for bb in range(nb):
                L = int(ncap[bb, t, r])
                piece_col0[bb, t, r] = npieces
                if L == 0:
                    continue
                pos = int(seg_pos[bb, t, r])
                c0, c1 = pos // P, (pos + L - 1) // P
                for chunk in range(c0, c1 + 1):
                    ci = int(chunk2call[chunk])
                    submm[t][r].append(
                        (ci, chunk - calls[ci]["chunk0"], chunk, npieces))
                    npieces += 1
    tile_pcol0[tiles] = npieces
    meta.submm = submm
    meta.piece_col0 = piece_col0
    meta.tile_pcol0 = tile_pcol0
    meta.npieces = npieces
    return meta


def _prep_core_arrays(meta, src, dst, et, coef, npad, c):
    """Per-core idx/dstloc/coef arrays in the shared slot layout."""
    shard, tiles, nb = meta.shard, meta.tiles, meta.nb
    core = dst // shard
    m = core == c
    s, d, r, cf = src[m], dst[m], et[m], coef[m]
    b = s // BANK
    t_loc = (d % shard) // P

    order = np.lexsort((s, r, t_loc, b))
    s, d, r, cf, b, t_loc = (a[order] for a in (s, d, r, cf, b, t_loc))

    gid = (b * tiles + t_loc) * R + r  # bank-major group id, sorted order
    # rank within group
    n = s.shape[0]
    grp_change = np.concatenate([[True], gid[1:] != gid[:-1]])
    grp_start = np.maximum.accumulate(np.where(grp_change, np.arange(n), 0))
    rank = np.arange(n) - grp_start

    slot = meta.seg_pos.reshape(-1)[gid] + rank

    idx16 = np.zeros((16, meta.S // 16), np.int16)
    idx16[slot % 16, slot // 16] = (s % BANK).astype(np.int16)
    idx_arr = np.tile(idx16, (8, 1))

    # per-piece poisoned dst columns: rows not in the piece keep 999
    # (no iota match -> zero Sel entry); coef is per chunk.
    pc0 = meta.piece_col0.transpose(1, 2, 0).reshape(-1)  # (t,r,b) order
    gid_trb = (t_loc * R + r) * nb + b
    pcol = (pc0[gid_trb]
            + slot // P - meta.seg_pos.reshape(-1)[gid] // P)
    dstl = np.full((P, max(meta.npieces, 1), 1), 999.0, ml_dtypes.bfloat16)
    dstl[slot % P, pcol, 0] = (d % P).astype(ml_dtypes.bfloat16)
    coefa = np.zeros((P, meta.C, 1), ml_dtypes.bfloat16)
    coefa[slot % P, slot // P, 0] = cf.astype(ml_dtypes.bfloat16)
    return idx_arr, dstl, coefa


# ------------------------------------------------------------ program builder

def _build_program(meta1, meta2, npad, nqueries_per_core):
    shard, tiles = meta1.shard, meta1.tiles
    dtt = DT_TABLE

    nc = bacc.Bacc("TRN2", target_bir_lowering=False, debug=False,
                   num_swdge_queues=4)

    xpad = nc.declare_dram_parameter("xpad", [npad, F], dtt, isOutput=False)
    xshard = nc.declare_dram_parameter("xshard", [shard, F], dtt, isOutput=False)
    idx_ds, dstl_ds, coef_ds = [], [], []
    for li, meta in ((1, meta1), (2, meta2)):
        idx_ds.append(nc.declare_dram_parameter(
            f"idx{li}", [P, meta.S // 16], mybir.dt.int16, isOutput=False))
        dstl_ds.append(nc.declare_dram_parameter(
            f"dstl{li}", [P, max(meta.npieces, 1), 1], dtt, isOutput=False))
        coef_ds.append(nc.declare_dram_parameter(
            f"coef{li}", [P, meta.C, 1], dtt, isOutput=False))
    iota_d = nc.declare_dram_parameter("iota", [P, 1, P], dtt, isOutput=False)
    w1_d = nc.declare_dram_parameter("w1", [F, R * F], dtt, isOutput=False)
    w1r_d = nc.declare_dram_parameter("w1r", [F, F], dtt, isOutput=False)
    b1b_d = nc.declare_dram_parameter("b1b", [P, F], F32, isOutput=False)
    w2_d = nc.declare_dram_parameter("w2", [F, R * F], dtt, isOutput=False)
    w2r_d = nc.declare_dram_parameter("w2r", [F, F], dtt, isOutput=False)
    b2b_d = nc.declare_dram_parameter("b2b", [P, F], F32, isOutput=False)
    wfcn_d = nc.declare_dram_parameter("wfcn", [F, HID], dtt, isOutput=False)
    wfcf_d = nc.declare_dram_parameter("wfcf", [F, HID], dtt, isOutput=False)
    bfc_d = nc.declare_dram_parameter("bfc", [P, HID // P], F32, isOutput=False)
    nest_d = nc.declare_dram_parameter("nest", [nqueries_per_core, 1], mybir.dt.int32, isOutput=False)
    food_d = nc.declare_dram_parameter("food", [nqueries_per_core, 1], mybir.dt.int32, isOutput=False)
    out_d = nc.declare_dram_parameter("out", [HID, nqueries_per_core], F32, isOutput=True)

    AGCH = AGCH_FOR(tiles)
    tpc = tiles // AGCH      # tiles per AG chunk
    rpc = tpc * P            # rows per AG chunk (per core)
    h1sg = [nc.dram_tensor(f"h1s{g}", [rpc, F], dtt) for g in range(AGCH)]
    h1f = nc.dram_tensor("h1f", [npad, F], dtt)
    h2sg = [nc.dram_tensor(f"h2s{g}", [rpc, F], dtt) for g in range(AGCH)]
    h2f = nc.dram_tensor("h2f", [npad, F], dtt)

    with tile.TileContext(nc) as tc:
        with (
            tc.tile_pool(name="const", bufs=1) as cpool,
            tc.tile_pool(name="xg", bufs=16) as xgp,
            tc.tile_pool(name="sel", bufs=4) as selp,
            tc.tile_pool(name="work", bufs=3) as work,
            tc.tile_pool(name="pa", bufs=2, space="PSUM") as psum_a,
            tc.tile_pool(name="pt", bufs=2, space="PSUM") as psum_t,
            tc.tile_pool(name="po", bufs=2, space="PSUM") as psum_o,
        ):
            idx_ts, dstl_ts, coef_ts = [], [], []
            for li, meta in ((0, meta1), (1, meta2)):
                idx_ts.append(cpool.tile([P, meta.S // 16], mybir.dt.int16,
                                         name=f"idx{li}", tag=f"idx{li}"))
                dstl_ts.append(cpool.tile([P, max(meta.npieces, 1), 1], dtt,
                                          name=f"dstl{li}", tag=f"dstl{li}"))
                coef_ts.append(cpool.tile([P, meta.C, 1], dtt,
                                          name=f"coef{li}", tag=f"coef{li}"))
            iota_t = cpool.tile([P, 1, P], dtt)
            w1_t = cpool.tile([F, R * F], dtt)
            w1r_t = cpool.tile([F, F], dtt)
            b1b_t = cpool.tile([P, F], F32)
            w2_t = cpool.tile([F, R * F], dtt)
            w2r_t = cpool.tile([F, F], dtt)
            b2b_t = cpool.tile([P, F], F32)
            wfcn_t = cpool.tile([F, HID], dtt)
            wfcf_t = cpool.tile([F, HID], dtt)
            bfc_t = cpool.tile([P, HID // P], F32)
            nest_t = cpool.tile([nqueries_per_core, 1], mybir.dt.int32)
            food_t = cpool.tile([nqueries_per_core, 1], mybir.dt.int32)
            ident = cpool.tile([P, P], dtt)
            zeros_t = cpool.tile([P, F], F32)
            nc.gpsimd.memset(zeros_t[:], 0.0)

            for tt, dd in [(idx_ts[0], idx_ds[0]), (dstl_ts[0], dstl_ds[0]),
                           (coef_ts[0], coef_ds[0]), (idx_ts[1], idx_ds[1]),
                           (dstl_ts[1], dstl_ds[1]), (coef_ts[1], coef_ds[1]),
                           (iota_t, iota_d), (w1_t, w1_d), (w1r_t, w1r_d),
                           (b1b_t, b1b_d), (w2_t, w2_d), (w2r_t, w2r_d),
                           (b2b_t, b2b_d), (wfcn_t, wfcn_d), (wfcf_t, wfcf_d),
                           (bfc_t, bfc_d), (nest_t, nest_d), (food_t, food_d)]:
                nc.sync.dma_start(out=tt[:], in_=dd[:])
            make_identity(nc, ident[:])

            qctr = [0]

            def layer(meta, idx_t, dstl_t, coef_t,
                      table_h, root_fn, w_t, wr_t, bb_t, relu, outg, hf):
                xg_tiles = {}

                def emit_call(ci):
                    call = meta.calls[ci]
                    ncol = call["n"] // P
                    xt = xgp.tile([P, CALL_MAX // P, F], dtt, tag="xg")
                    nc.gpsimd.dma_gather(
                        xt[:, :ncol, :],
                        table_h[call["row0"]: call["row0"] + call["rows"]],
                        idx_t[:, call["col0"]: call["col0"] + call["n"] // 16],
                        call["n"], call["n"], F,
                        queue_num=qctr[0] % 4,
                    )
                    qctr[0] += 1
                    # fold mean-normalization coefs into the gathered rows
                    c0 = call["chunk0"]
                    nc.vector.tensor_tensor(
                        out=xt[:, :ncol, :], in0=xt[:, :ncol, :],
                        in1=coef_t[:, c0:c0 + ncol, :].to_broadcast([P, ncol, F]),
                        op=mybir.AluOpType.mult,
                    )
                    xg_tiles[ci] = xt

                for t in range(tiles):
                    # one batched Sel build for all of this tile's pieces
                    k0, k1 = int(meta.tile_pcol0[t]), int(meta.tile_pcol0[t + 1])
                    kt = k1 - k0
                    if kt > 0:
                        selb = selp.tile([P, max(kt, 1), P], dtt, tag="sel")
                        nc.vector.tensor_tensor(
                            out=selb[:, :kt, :],
                            in0=dstl_t[:, k0:k1, :].to_broadcast([P, kt, P]),
                            in1=iota_t[:].to_broadcast([P, kt, P]),
                            op=mybir.AluOpType.is_equal,
                        )
                    at_p = psum_a.tile([P, R * F], F32, space="PSUM", tag="pa")
                    used_r = []
                    for r in range(R):
                        pieces = meta.submm[t][r]
                        if not pieces:
                            continue
                        used_r.append(r)
                        for i, (ci, ccol, chunk, pcol) in enumerate(pieces):
                            if ci not in xg_tiles:
                                emit_call(ci)
                            nc.tensor.matmul(
                                out=at_p[:, r * F:(r + 1) * F],
                                lhsT=xg_tiles[ci][:, ccol, :],
                                rhs=selb[:, pcol - k0, :],
                                start=(i == 0), stop=(i == len(pieces) - 1),
                            )
                    at_s = work.tile([P, R * F], dtt, tag="at_s")
                    if used_r:
                        nc.vector.tensor_copy(out=at_s[:], in_=at_p[:])
                    # root rows, transposed via PE
                    xd = work.tile([P, F], dtt, tag="xd")
                    nc.sync.dma_start(out=xd[:], in_=root_fn(t))
                    xt_p = psum_t.tile([P, F], dtt, space="PSUM", tag="pt")
                    nc.tensor.transpose(out=xt_p[:], in_=xd[:], identity=ident[:])
                    xts = work.tile([P, F], dtt, tag="xts")
                    nc.vector.tensor_copy(out=xts[:], in_=xt_p[:])

                    out_p = psum_o.tile([P, F], F32, space="PSUM", tag="po")
                    mms = [(at_s[:, r * F:(r + 1) * F], w_t[:, r * F:(r + 1) * F])
                           for r in used_r] + [(xts[:], wr_t[:])]
                    for i, (l, rr) in enumerate(mms):
                        nc.tensor.matmul(out=out_p[:], lhsT=l, rhs=rr,
                                         start=(i == 0), stop=(i == len(mms) - 1))
                    ht = work.tile([P, F], dtt, tag="ht")
                    if relu:
                        hsum = work.tile([P, F], F32, tag="hsum")
                        nc.vector.tensor_tensor(out=hsum[:], in0=out_p[:],
                                                in1=bb_t[:], op=mybir.AluOpType.add)
                        nc.vector.tensor_tensor(out=ht[:], in0=hsum[:],
                                                in1=zeros_t[:], op=mybir.AluOpType.max)
                    else:
                        nc.vector.tensor_tensor(out=ht[:], in0=out_p[:],
                                                in1=bb_t[:], op=mybir.AluOpType.add)
                    g, tl = t // tpc, t % tpc
                    nc.sync.dma_start(out=outg[g][tl * P:(tl + 1) * P], in_=ht[:])
                    if tl == tpc - 1:
                        # this AG chunk's rows are complete: fire it now so it
                        # overlaps the remaining tiles' compute.  Output is a
                        # contiguous block of the permuted-layout full table
                        # (row order: chunk-major, then core, then local row).
                        nc.gpsimd.collective_compute(
                            "AllGather", mybir.AluOpType.bypass,
                            replica_groups=[list(range(NC))],
                            ins=[outg[g][:, :]],
                            outs=[hf[g * NC * rpc:(g + 1) * NC * rpc, :]],
                        )

            layer(meta1, idx_ts[0], dstl_ts[0], coef_ts[0],
                  xpad, lambda t: xshard[t * P:(t + 1) * P], w1_t, w1r_t,
                  b1b_t, True, h1sg, h1f)
            layer(meta2, idx_ts[1], dstl_ts[1], coef_ts[1],
                  h1f, lambda t: h1sg[t // tpc][(t % tpc) * P:(t % tpc + 1) * P],
                  w2_t, w2r_t, b2b_t, False, h2sg, h2f)

            # final MLP: gather query rows, transpose, two-chunk matmul
            for name, qt in [("nest", nest_t), ("food", food_t)]:
                g = work.tile([nqueries_per_core, F], dtt, tag=f"q_{name}")
                nc.gpsimd.indirect_dma_start(
                    out=g[:], out_offset=None, in_=h2f[:],
                    in_offset=bass.IndirectOffsetOnAxis(ap=qt[:, :1], axis=0),
                )
                gp = psum_t.tile([P, nqueries_per_core], dtt, space="PSUM", tag="pt")
                nc.tensor.transpose(out=gp[:, :], in_=g[:],
                                    identity=ident[:nqueries_per_core, :nqueries_per_core])
                gs = work.tile([P, nqueries_per_core], dtt, tag=f"qT_{name}")
                nc.vector.tensor_copy(out=gs[:], in_=gp[:])
                if name == "nest":
                    nestT = gs
                else:
                    foodT = gs
            for c in range(HID // P):
                fcp = psum_o.tile([P, nqueries_per_core], F32, space="PSUM", tag="po")
                nc.tensor.matmul(out=fcp[:], lhsT=wfcn_t[:, c * P:(c + 1) * P],
                                 rhs=nestT[:], start=True, stop=False)
                nc.tensor.matmul(out=fcp[:], lhsT=wfcf_t[:, c * P:(c + 1) * P],
                                 rhs=foodT[:], start=False, stop=True)
                fco = work.tile([P, nqueries_per_core], F32, tag="fco")
                nc.vector.tensor_scalar(out=fco[:], in0=fcp[:],
                                        scalar1=bfc_t[:, c:c + 1], scalar2=0.0,
                                        op0=mybir.AluOpType.add,
                                        op1=mybir.AluOpType.max)
                nc.sync.dma_start(out=out_d[c * P:(c + 1) * P, :], in_=fco[:])

    nc.compile()
    return nc


# ------------------------------------------------------------------- kernel()

def _run(x, edge_index, edge_type, nest_idx, food_idx,
         W_rel1, W_root1, b1, W_rel2, W_root2, b2, W_fc, b_fc,
         n_nodes, npad, nb_total):
    global _last_exec_ns
    shard = npad // NC
    src = np.asarray(edge_index[0], np.int64)
    dst = np.asarray(edge_index[1], np.int64)
    et = np.asarray(edge_type, np.int64)
    nest_idx = np.asarray(nest_idx, np.int64)
    food_idx = np.asarray(food_idx, np.int64)

    key = dst * R + et
    deg = np.bincount(key, minlength=n_nodes * R)
    coef = (1.0 / np.maximum(deg, 1.0)[key]).astype(np.float32)

    meta1 = _prep_meta(src, dst, et, npad, n_nodes)
    shard_, tiles_ = meta1.shard, meta1.tiles
    agch = AGCH_FOR(tiles_)
    tpc_, rpc_ = tiles_ // agch, (tiles_ // agch) * P

    def permrow(v):
        c = v // shard_
        l = v % shard_
        return (l // rpc_) * NC * rpc_ + c * rpc_ + (l % rpc_)

    src2 = permrow(src)
    meta2 = _prep_meta(src2, dst, et, npad, n_nodes)

    x_pad = np.zeros((npad, F), NP_TABLE)
    x_pad[:n_nodes] = np.asarray(x, np.float32).astype(NP_TABLE)
    iota = np.tile(np.arange(P, dtype=np.float32), (P, 1)).astype(
        ml_dtypes.bfloat16).reshape(P, 1, P)
    w1 = np.concatenate([np.asarray(W_rel1[r], np.float32) for r in range(R)], axis=1).astype(NP_TABLE)
    w2 = np.concatenate([np.asarray(W_rel2[r], np.float32) for r in range(R)], axis=1).astype(NP_TABLE)
    w1r = np.asarray(W_root1, np.float32).astype(NP_TABLE)
    w2r = np.asarray(W_root2, np.float32).astype(NP_TABLE)
    b1b = np.tile(np.asarray(b1, np.float32), (P, 1))
    b2b = np.tile(np.asarray(b2, np.float32), (P, 1))
    wfc = np.asarray(W_fc, np.float32)
    wfcn = wfc[:F].astype(NP_TABLE)
    wfcf = wfc[F:].astype(NP_TABLE)
    bfc = np.asarray(b_fc, np.float32).reshape(HID // P, P).T.copy()

    nq = nb_total // NC
    in_maps = []
    for c in range(NC):
        idx1, dstl1, coef1 = _prep_core_arrays(meta1, src, dst, et, coef, npad, c)
        idx2, dstl2, coef2 = _prep_core_arrays(meta2, src2, dst, et, coef, npad, c)
        in_maps.append(dict(
            xpad=x_pad,
            xshard=np.ascontiguousarray(x_pad[c * shard:(c + 1) * shard]),
            idx1=idx1, dstl1=dstl1, coef1=coef1,
            idx2=idx2, dstl2=dstl2, coef2=coef2, iota=iota,
            w1=w1, w1r=w1r, b1b=b1b, w2=w2, w2r=w2r, b2b=b2b,
            wfcn=wfcn, wfcf=wfcf, bfc=bfc,
            nest=permrow(nest_idx[c * nq:(c + 1) * nq]).astype(np.int32)[:, None],
            food=permrow(food_idx[c * nq:(c + 1) * nq]).astype(np.int32)[:, None],
        ))

    nc = _build_program(meta1, meta2, npad, nq)

    trace = bool(os.environ.get("KERNEL_PROFILE"))
    res = run_bass_kernel_spmd(nc, in_maps, list(range(NC)), trace=trace)
    if trace:
        _last_exec_ns = res.exec_time_ns

    out = np.empty((nb_total, HID), np.float32)
    for c in range(NC):
        out[c * nq:(c + 1) * nq] = res.results[c]["out"].T
    return out


def kernel(x, edge_index, edge_type, nest_idx, food_idx,
           W_rel1, W_root1, b1, W_rel2, W_root2, b2, W_fc, b_fc):
    return _run(x, edge_index, edge_type, nest_idx, food_idx,
                W_rel1, W_root1, b1, W_rel2, W_root2, b2, W_fc, b_fc,
                n_nodes=N_FULL, npad=NPAD_FULL, nb_total=B_FULL)


# ------------------------------------------------------------------- selftest

def _np_reference(x, src, dst, et, nest, food, W_rel1, W_root1, b1,
                  W_rel2, W_root2, b2, W_fc, b_fc, n):
    def conv(h, W_rel, W_root, b):
        hr = np.einsum("nf,rfo->nro", h, W_rel)
        msgs = hr[src, et]
        key = dst * R + et
        deg = np.bincount(key, minlength=n * R).astype(np.float32)
        norm = 1.0 / np.maximum(deg, 1.0)
        out = np.zeros((n, W_rel.shape[2]), np.float32)
        np.add.at(out, dst, msgs * norm[key][:, None])
        return out + h @ W_root + b

    h = np.maximum(conv(x, W_rel1, W_root1, b1), 0.0)
    h = conv(h, W_rel2, W_root2, b2)
    comb = np.concatenate([h[nest], h[food]], axis=1)
    return np.maximum(comb @ W_fc + b_fc, 0.0)


def _selftest():
    global BANK
    BANK = 1024  # exercise multi-bank path at small scale
    n, npad, nb = 4000, 4096, 256
    e = 32768
    rng = np.random.default_rng(0)
    x = rng.standard_normal((n, F)).astype(np.float32)
    ei = rng.integers(0, n, (2, e)).astype(np.int64)
    et = rng.integers(0, R, e).astype(np.int64)
    nest = rng.integers(0, n, nb).astype(np.int64)
    food = rng.integers(0, n, nb).astype(np.int64)
    s1, s2, sf = 1 / np.sqrt(F), 1 / np.sqrt(F), 1 / np.sqrt(2 * F)
    W_rel1 = (rng.standard_normal((R, F, F)) * s1).astype(np.float32)
    W_root1 = (rng.standard_normal((F, F)) * s1).astype(np.float32)
    b1 = rng.standard_normal(F).astype(np.float32) * 0.1
    W_rel2 = (rng.standard_normal((R, F, F)) * s2).astype(np.float32)
    W_root2 = (rng.standard_normal((F, F)) * s2).astype(np.float32)
    b2 = rng.standard_normal(F).astype(np.float32) * 0.1
    W_fc = (rng.standard_normal((2 * F, HID)) * sf).astype(np.float32)
    b_fc = rng.standard_normal(HID).astype(np.float32) * 0.1

    exp = _np_reference(x, ei[0], ei[1], et, nest, food, W_rel1, W_root1, b1,
                        W_rel2, W_root2, b2, W_fc, b_fc, n)
    act = _run(x, ei, et, nest, food, W_rel1, W_root1, b1,
               W_rel2, W_root2, b2, W_fc, b_fc,
               n_nodes=n, npad=npad, nb_total=nb)
    denom = np.abs(exp).max()
    err = np.abs(act - exp).max() / denom
    print(f"selftest: rel_err={err:.2e} (absmax denom {denom:.3f})")
    assert err < 2e-2, "selftest FAILED"
    print("SELFTEST PASSED")


if __name__ == "__main__":
    _selftest()
